# revision 28
# baseline (speedup 1.0000x reference)
"""AdaptiveSoftmax (loss_fn) on 8 TRN2 NeuronCores.

Strategy: data-parallel over the token dim N=2048 -> 256 rows/core.
Each core computes its [256, 50257] slice of probs plus the per-row
cluster probabilities (aux) needed to reconstruct the scalar loss on
host from gathered outputs.

Device program (SPMD, identical on all cores), per 128-row tile:
  root:  e = exp(logitsT.T @ head_w) (bf16 in, fp32 psum, K=1024) with
         fused row-sum -> head probs = e/s, clust_prob = e[:,2000:2002]/s
  tail0: h0 = logits' @ [pw0; pb0]; e0 = exp(h0T.T @ sw0 + sb0) stored
         with row sums; output = e0*A + B on DVE, A = m0*clust0/s0,
         B = (1-m0)*clust0/8000; one 4MB DMA per row-tile.
  tail1: two passes (exp-sum, then fused out = exp(tlog*m1 + bias),
         bias = ln(clust1) - m1*ln(s1) - (1-m1)*ln(40257)), recomputing
         tlog to avoid storing the 20MB intermediate. When sb1 == 0 the
         [64, 40257] weight matrix is packed as two column halves on the
         full 128 partitions (K=64, h1T duplicated on both halves) --
         halves its SBUF footprint and fixes its DMA parallelism; a
         65-row layout with the bias folded as a weight row (ones row in
         h1T) is the fallback. ACT program order weaves tail1 pass2
         (DMA-bound) with the next row-tile's pass1 (compute-only) so
         the scalar engine never idles on the output stream.

Perf notes: inputs are packed on host so each tensor is one or few
DMAs ([128, k, cols] K-major layouts), ordered so root's weights land
first; output DMA issue alternates between the sync and gpsimd
sequencers and outputs are staged 3 ACT-blocks (4608 cols) wide for
better DMA efficiency (measured 305 vs 275 GB/s/core). exp skips
max-subtraction: all softmax inputs stay within +-4 (0.02-scaled
weights) where fp32 exp is 2-ULP exact. ACT ops run on 1536-col
(3 PSUM bank) blocks; pass-1 exp dumps in place into its own PSUM
block. Root weights and the stored e0 share one SBUF region via nested
pools (root finishes before tail0's sums are consumed).
"""
import sys

sys.path.insert(0, "/opt/trn_rl_repo")
import numpy as np

N_CORES = 8
N_TOK, D, V = 2048, 1024, 50257
ROWS = N_TOK // N_CORES          # 256 rows per core
P = 128                          # partitions
RT = ROWS // P                   # 2 row tiles per core
CUT0, CUT1 = 2000, 10000
H_UNITS = 2002                   # head width (2000 + 2 cluster cols)
T0_SIZE = CUT1 - CUT0            # 8000
T1_SIZE = V - CUT1               # 40257
T1_VPAD = 79 * 512               # 40448: pass-1 col coverage (incl pad tail)
T1_PAD = 80 * 512                # 40960: DRAM padding of w1s
NEG_BIG = -1.0e30                # pad-col bias so exp() there is exactly 0
BLK = 1536                       # ACT/psum block width (3 PSUM banks)
T1_HALF = 20480                  # packed-w1s column split (512-aligned)


def _blocks(total, blk=BLK):
    out, c = [], 0
    while c < total:
        out.append((c, min(blk, total - c)))
        c += blk
    return out


def _build(reps=1, phases=("root", "t0", "t1"), has_pb=True, has_sb0=True, has_sb1=True):
    import concourse.bacc as bacc
    import concourse.mybir as mybir
    from concourse import tile
    from concourse.masks import make_identity

    F32 = mybir.dt.float32
    BF16 = mybir.dt.bfloat16

    nc = bacc.Bacc(None, target_bir_lowering=False, debug=True)
    lgm = nc.declare_dram_parameter("lgm", [P, 8, ROWS], BF16, isOutput=False)
    whm = nc.declare_dram_parameter("whm", [P, 8, H_UNITS], BF16, isOutput=False)
    w0pm = nc.declare_dram_parameter("w0pm", [P, 8, 256], BF16, isOutput=False)
    w0pb = nc.declare_dram_parameter("w0pb", [1, 256], BF16, isOutput=False)
    w0sm = nc.declare_dram_parameter("w0sm", [P, 2, T0_SIZE], BF16, isOutput=False)
    w0b = nc.declare_dram_parameter("w0b", [1, T0_SIZE], BF16, isOutput=False)
    w1pm = nc.declare_dram_parameter("w1pm", [P, 8, 64], BF16, isOutput=False)
    w1pb = nc.declare_dram_parameter("w1pb", [1, 64], BF16, isOutput=False)
    if has_sb1:
        w1s = nc.declare_dram_parameter("w1s", [65, T1_PAD], BF16, isOutput=False)
    else:
        # sb1 == 0: two column halves packed onto the full 128 partitions
        w1s = nc.declare_dram_parameter("w1s", [P, T1_HALF], BF16, isOutput=False)
    mkm = nc.declare_dram_parameter("mkm", [P, 8], F32, isOutput=False)
    probs = nc.declare_dram_parameter("probs", [ROWS, V], F32, isOutput=True)
    aux = nc.declare_dram_parameter("aux", [ROWS, 2], F32, isOutput=True)

    with tile.TileContext(nc) as tc:
        for _rep in range(reps):
            _trace_body(nc, tc, mybir, make_identity,
                        lgm, whm, w0pm, w0pb, w0sm, w0b, w1pm, w1pb, w1s, mkm,
                        probs, aux, phases, has_pb, has_sb0, has_sb1)
    nc.compile()
    return nc


def _trace_body(nc, tc, mybir, make_identity,
                lgm, whm, w0pm, w0pb, w0sm, w0b, w1pm, w1pb, w1s, mkm,
                probs, aux, phases, has_pb, has_sb0, has_sb1):
    F32 = mybir.dt.float32
    BF16 = mybir.dt.bfloat16
    Exp = mybir.ActivationFunctionType.Exp
    Ln = mybir.ActivationFunctionType.Ln
    Alu = mybir.AluOpType
    AX = mybir.AxisListType
    import contextlib

    out_eng = [nc.sync, nc.gpsimd]   # alternate output-DMA issuing engine
    out_i = [0]

    def out_dma(dst, src):
        out_eng[out_i[0] % 2].dma_start(out=dst, in_=src)
        out_i[0] += 1

    with (
        contextlib.nullcontext(tc),
        tc.tile_pool(name="const", bufs=1) as constp,
        tc.tile_pool(name="lg", bufs=1) as lgp,
        tc.tile_pool(name="ll", bufs=1) as llp,
        tc.tile_pool(name="wpool", bufs=1) as wpool,
        tc.tile_pool(name="acc", bufs=2) as accp,
        tc.tile_pool(name="rowv", bufs=2) as rowp,
        tc.tile_pool(name="er", bufs=2 if not has_sb1 else 1) as erp,
        tc.tile_pool(name="hsc", bufs=2) as hscp,
        tc.tile_pool(name="stage", bufs=3 if not has_sb1 else 2) as stagep,
        tc.tile_pool(name="ps_main", bufs=2, space="PSUM") as psm,
        tc.tile_pool(name="ps_h", bufs=1, space="PSUM") as psh,
        tc.tile_pool(name="ps_t", bufs=1, space="PSUM") as pst,
    ):
        ident = constp.tile([P, P], BF16, tag="ident")
        make_identity(nc, ident[:])
        ones_row = constp.tile([1, ROWS], BF16, tag="ones_row")
        nc.vector.memset(ones_row[:], 1.0)

        lg = lgp.tile([P, 8, ROWS], BF16, tag="lg")
        nc.gpsimd.dma_start(out=lg[:, :, :], in_=lgm[:, :, :])
        whp_ctx = tc.tile_pool(name="whp", bufs=1)
        whp = whp_ctx.__enter__()
        wh_t = None
        if "root" in phases:
            wh_t = whp.tile([P, 8, H_UNITS], BF16, tag="wh")
            # split so the first root block's columns land ~10us earlier
            nc.gpsimd.dma_start(out=wh_t[:, :, 0:BLK], in_=whm[:, :, 0:BLK])
            nc.gpsimd.dma_start(out=wh_t[:, :, BLK:], in_=whm[:, :, BLK:])

        mk_t = llp.tile([P, 8], F32, tag="mk")
        nc.gpsimd.dma_start(out=mk_t[:, :], in_=mkm[:, :])
        m_t = {}
        for j, nm in enumerate(("m0", "m0c", "m1", "m1c")):
            for rt in range(RT):
                m_t[(nm, rt)] = mk_t[:, j * RT + rt:j * RT + rt + 1]
        clust = [llp.tile([P, 2], F32, tag=f"clust{rt}", name=f"clust{rt}") for rt in range(RT)]
        lncl = [llp.tile([P, 2], F32, tag=f"lncl{rt}", name=f"lncl{rt}") for rt in range(RT)]

        if "root" not in phases:
            for rt in range(RT):
                nc.vector.memset(clust[rt][:, :], 0.25)
                nc.vector.memset(lncl[rt][:, :], -1.4)

        # non-root weights (root's wh was DMA'd first, above)
        w0p_t = w0pb_t = w0s_t = w0b_t = None
        w1p_t = w1pb_t = w1s_t = None
        if "t0" in phases:
            w0p_t = wpool.tile([P, 8, 256], BF16, tag="w0p")
            nc.gpsimd.dma_start(out=w0p_t[:, :, :], in_=w0pm[:, :, :])
            w0s_t = wpool.tile([P, 2, T0_SIZE], BF16, tag="w0s")
            nc.gpsimd.dma_start(out=w0s_t[:, :, :], in_=w0sm[:, :, :])
            if has_pb:
                w0pb_t = wpool.tile([1, 256], BF16, tag="w0pb")
                nc.gpsimd.dma_start(out=w0pb_t[:, :], in_=w0pb[0:1, :])
            if has_sb0:
                w0b_t = wpool.tile([1, T0_SIZE], BF16, tag="w0b")
                nc.gpsimd.dma_start(out=w0b_t[:, :], in_=w0b[0:1, :])
        if "t1" in phases:
            w1p_t = wpool.tile([P, 8, 64], BF16, tag="w1p")
            nc.gpsimd.dma_start(out=w1p_t[:, :, :], in_=w1pm[:, :, :])
            if has_pb:
                w1pb_t = wpool.tile([1, 64], BF16, tag="w1pb")
                nc.gpsimd.dma_start(out=w1pb_t[:, :], in_=w1pb[0:1, :])
            if has_sb1:
                w1s_t = wpool.tile([65, T1_PAD], BF16, tag="w1s")
                for q in range(4):
                    nc.gpsimd.dma_start(out=w1s_t[:, q * 10240:(q + 1) * 10240],
                                        in_=w1s[:, q * 10240:(q + 1) * 10240])
            else:
                w1s_t = wpool.tile([P, T1_HALF], BF16, tag="w1s")
                nc.gpsimd.dma_start(out=w1s_t[:, :], in_=w1s[:, :])

        def project(rt, width, wp_t, wpb_t, htag):
            """h = logits' @ wp' -> transposed bf16 K-chunk tiles."""
            rs = slice(rt * P, (rt + 1) * P)
            ph = psh.tile([P, 256], F32, tag="ph")
            for k in range(8):
                nc.tensor.matmul(ph[:, :width], lg[:, k, rs], wp_t[:, k, :],
                                 start=(k == 0), stop=(k == 7 and not has_pb))
            if has_pb:
                nc.tensor.matmul(ph[:, :width], ones_row[:, rs], wpb_t[:, :],
                                 start=False, stop=True)
            h = hscp.tile([P, width], BF16, tag=f"h_{htag}")
            nc.vector.tensor_copy(h[:, :], ph[:, :width])
            parts = []
            for c in range(0, width, P):
                cw = min(P, width - c)
                ptt = pst.tile([P, P], BF16, tag="ptt")
                nc.tensor.transpose(ptt[:cw, :], h[:, c:c + cw], ident[:])
                ht = hscp.tile([P, P], BF16, tag=f"hT_{htag}{c // P}")
                nc.vector.tensor_copy(ht[:cw, :], ptt[:cw, :])
                parts.append((ht, cw))
            return parts

        # ---------------- root (nested pool: space reused by e0) ----------
        if True:
            try:
                blocks = _blocks(H_UNITS)
                for rt in (range(RT) if "root" in phases else ()):
                    rs = slice(rt * P, (rt + 1) * P)
                    e_root = erp.tile([P, H_UNITS], F32, tag="e_root")
                    acc_r = accp.tile([P, len(blocks)], F32, tag="acc_r")
                    for bi, (b0, bn) in enumerate(blocks):
                        pt = psm.tile([P, BLK], F32, tag="pm")
                        for s0 in range(0, bn, 512):
                            scn = min(512, bn - s0)
                            for k in range(8):
                                nc.tensor.matmul(
                                    pt[:, s0:s0 + scn], lg[:, k, rs],
                                    wh_t[:, k, b0 + s0:b0 + s0 + scn],
                                    start=(k == 0), stop=(k == 7),
                                )
                        nc.scalar.activation(e_root[:, b0:b0 + bn], pt[:, :bn], Exp,
                                             accum_out=acc_r[:, bi:bi + 1])
                    s_r = rowp.tile([P, 1], F32, tag="s_r")
                    nc.vector.tensor_reduce(s_r[:, :], acc_r[:, :], axis=AX.X, op=Alu.add)
                    rec_r = rowp.tile([P, 1], F32, tag="rec_r")
                    nc.vector.reciprocal(rec_r[:, :], s_r[:, :])
                    nc.vector.tensor_scalar(
                        clust[rt][:, :], e_root[:, CUT0:CUT0 + 2], rec_r[:, :], None, Alu.mult
                    )
                    nc.scalar.activation(lncl[rt][:, :], clust[rt][:, :], Ln)
                    out_dma(aux[rs, :], clust[rt][:, :])
                    nc.vector.tensor_scalar(e_root[:, 0:CUT0], e_root[:, 0:CUT0],
                                            rec_r[:, :], None, Alu.mult)
                    out_dma(probs[rs, 0:CUT0], e_root[:, 0:CUT0])
            finally:
                whp_ctx.__exit__(None, None, None)

        # ---------------- tails ----------------
        with tc.tile_pool(name="e0p", bufs=1) as e0p:
            def t0_ops(rt):
                """tail0: single exp pass into stored e0; pass2 on DVE."""
                rs = slice(rt * P, (rt + 1) * P)
                hT = project(rt, 256, w0p_t, w0pb_t, "t0")
                blocks = _blocks(T0_SIZE)
                e0 = e0p.tile([P, T0_SIZE], F32, tag="e0", name=f"e0_{rt}")
                acc = accp.tile([P, len(blocks)], F32, tag=f"a0_{rt}",
                                name=f"a0_{rt}")

                def p1_op(bi, b0, bn):
                    def op():
                        pt = psm.tile([P, BLK], F32, tag="pm", name="pt")
                        for s0 in range(0, bn, 512):
                            scn = min(512, bn - s0)
                            c0 = b0 + s0
                            nc.tensor.matmul(pt[:, s0:s0 + scn], hT[0][0][:, :],
                                             w0s_t[:, 0, c0:c0 + scn],
                                             start=True, stop=False)
                            nc.tensor.matmul(pt[:, s0:s0 + scn], hT[1][0][:, :],
                                             w0s_t[:, 1, c0:c0 + scn],
                                             start=False, stop=not has_sb0)
                            if has_sb0:
                                nc.tensor.matmul(pt[:, s0:s0 + scn], ones_row[:, 0:P],
                                                 w0b_t[:, c0:c0 + scn],
                                                 start=False, stop=True)
                        nc.scalar.activation(e0[:, b0:b0 + bn], pt[:, :bn], Exp,
                                             accum_out=acc[:, bi:bi + 1])
                    return op

                def p2():
                    s0_ = rowp.tile([P, 1], F32, tag="s0_", name="s0_")
                    nc.vector.tensor_reduce(s0_[:, :], acc[:, :], axis=AX.X, op=Alu.add)
                    rec0 = rowp.tile([P, 1], F32, tag="rec0", name="rec0")
                    nc.vector.reciprocal(rec0[:, :], s0_[:, :])
                    a0 = rowp.tile([P, 1], F32, tag="a0s", name="a0s")
                    nc.vector.tensor_scalar(a0[:, :], m_t[("m0", rt)],
                                            clust[rt][:, 0:1], rec0[:, :],
                                            Alu.mult, Alu.mult)
                    b0_ = rowp.tile([P, 1], F32, tag="b0s", name="b0s")
                    nc.vector.tensor_scalar(b0_[:, :], m_t[("m0c", rt)],
                                            clust[rt][:, 0:1], 1.0 / T0_SIZE,
                                            Alu.mult, Alu.mult)
                    nc.vector.tensor_scalar(e0[:, :], e0[:, :], a0[:, :], b0_[:, :],
                                            Alu.mult, Alu.add)
                    out_dma(probs[rs, CUT0:CUT1], e0[:, :])

                return [p1_op(bi, b0, bn) for bi, (b0, bn) in enumerate(blocks)], p2

            def t1_ops(rt):
                """tail1: two passes, tlog recomputed; pass2 fused on ACT."""
                rs = slice(rt * P, (rt + 1) * P)
                parts = project(rt, 64, w1p_t, w1pb_t, "t1")
                if has_sb1:
                    h1T = hscp.tile([65, P], BF16, tag=f"h1T_{rt}", name=f"h1T_{rt}")
                    nc.vector.tensor_copy(h1T[0:64, :], parts[0][0][0:64, :])
                    nc.vector.memset(h1T[64:65, :], 1.0)
                    p1_blocks = _blocks(T1_VPAD)
                else:
                    # duplicate h1T on both partition halves for the packed weights
                    h1T = hscp.tile([P, P], BF16, tag=f"h1T_{rt}", name=f"h1T_{rt}")
                    nc.vector.tensor_copy(h1T[0:64, :], parts[0][0][0:64, :])
                    nc.vector.tensor_copy(h1T[64:128, :], parts[0][0][0:64, :])
                    p1_blocks = _blocks(T1_SIZE)
                p2_blocks = _blocks(T1_SIZE)
                acc = accp.tile([P, len(p1_blocks)], F32, tag=f"a1_{rt}",
                                name=f"a1_{rt}")
                bb = rowp.tile([P, 1], F32, tag=f"bb_{rt}", name=f"bb_{rt}")

                def mm_block(pt, b0, bn):
                    for s0 in range(0, bn, 512):
                        scn = min(512, bn - s0)
                        c = b0 + s0
                        if has_sb1:
                            nc.tensor.matmul(pt[:, s0:s0 + scn], h1T[:, :],
                                             w1s_t[:, c:c + scn],
                                             start=True, stop=True)
                        else:
                            lo, qo = (0, c) if c < T1_HALF else (64, c - T1_HALF)
                            nc.tensor.matmul(pt[:, s0:s0 + scn],
                                             h1T[lo:lo + 64, :],
                                             w1s_t[lo:lo + 64, qo:qo + scn],
                                             start=True, stop=True)

                def p1_op(bi, b0, bn):
                    def op():
                        pt = psm.tile([P, BLK], F32, tag="pm", name="pt")
                        mm_block(pt, b0, bn)
                        nc.scalar.activation(pt[:, :bn], pt[:, :bn], Exp,
                                             accum_out=acc[:, bi:bi + 1])
                    return op

                def mid():
                    ln_t1 = float(np.log(np.float32(T1_SIZE)))
                    s1 = rowp.tile([P, 1], F32, tag="s1", name="s1")
                    nc.vector.tensor_reduce(s1[:, :], acc[:, :], axis=AX.X, op=Alu.add)
                    lns1 = rowp.tile([P, 1], F32, tag="lns1", name="lns1")
                    nc.scalar.activation(lns1[:, :], s1[:, :], Ln)
                    tm = rowp.tile([P, 1], F32, tag="tm", name="tm")
                    nc.vector.tensor_scalar(tm[:, :], m_t[("m1", rt)],
                                            lns1[:, :], None, Alu.mult)
                    tmc = rowp.tile([P, 1], F32, tag="tmc", name="tmc")
                    nc.vector.tensor_scalar(tmc[:, :], m_t[("m1c", rt)],
                                            ln_t1, None, Alu.mult)
                    nc.vector.tensor_tensor(bb[:, :], lncl[rt][:, 1:2], tm[:, :],
                                            Alu.subtract)
                    nc.vector.tensor_tensor(bb[:, :], bb[:, :], tmc[:, :], Alu.subtract)

                n_pair = 2 if has_sb1 else 3
                def p2_op(pair):
                    def op():
                        st = stagep.tile([P, n_pair * BLK], F32, tag="st", name="st")
                        off = 0
                        base = pair[0][0]
                        for b0, bn in pair:
                            pt = psm.tile([P, BLK], F32, tag="pm", name="pt")
                            mm_block(pt, b0, bn)
                            nc.scalar.activation(st[:, off:off + bn], pt[:, :bn], Exp,
                                                 bias=bb[:, :], scale=m_t[("m1", rt)])
                            off += bn
                        out_dma(probs[rs, CUT1 + base:CUT1 + base + off], st[:, :off])
                    return op

                pairs = [p2_blocks[i:i + n_pair] for i in range(0, len(p2_blocks), n_pair)]
                return ([p1_op(bi, b0, bn) for bi, (b0, bn) in enumerate(p1_blocks)],
                        mid,
                        [p2_op(pair) for pair in pairs])

            def weave(a_ops, b_ops):
                na, nb = len(a_ops), len(b_ops)
                n = max(na, nb)
                ia = ib = 0
                for i in range(n):
                    while ia * n < (i + 1) * na:
                        a_ops[ia]()
                        ia += 1
                    while ib * n < (i + 1) * nb:
                        b_ops[ib]()
                        ib += 1

            do_t0 = "t0" in phases
            do_t1 = "t1" in phases
            t0s = [t0_ops(rt) for rt in range(RT)] if do_t0 else []
            if do_t1:
                t1a_p1, t1a_mid, t1a_p2 = t1_ops(0)
                t1b_p1, t1b_mid, t1b_p2 = t1_ops(1)
            if do_t0:
                for op in t0s[0][0]:
                    op()
                t0s[0][1]()
            if do_t1:
                # rt1's tail0 pass1 hides inside tail1 rt0 pass1 (the shared
                # e0 buffer frees once rt0's 4MB output DMA drains)
                weave(t1a_p1, t0s[1][0] if do_t0 else [])
                if do_t0:
                    t0s[1][1]()
                t1a_mid()
                weave(t1a_p2, t1b_p1)
                t1b_mid()
                for op in t1b_p2:
                    op()
            elif do_t0:
                for op in t0s[1][0]:
                    op()
                t0s[1][1]()


def prep_in_maps(inputs):
    """Host-side prep shared by kernel() and bench: shard + pack + augment."""
    import ml_dtypes
    BF = ml_dtypes.bfloat16

    logits = np.asarray(inputs["logits"], np.float32)
    targets = np.asarray(inputs["targets"], np.int32)
    t0_pb = np.asarray(inputs["t0_pb"], np.float32)
    t1_pb = np.asarray(inputs["t1_pb"], np.float32)
    t0_sb = np.asarray(inputs["t0_sb"], np.float32)
    t1_sb_arr = np.asarray(inputs["t1_sb"], np.float32)
    has_pb = bool(np.any(t0_pb != 0) or np.any(t1_pb != 0))
    has_sb0 = bool(np.any(t0_sb != 0))
    has_sb1 = bool(np.any(t1_sb_arr != 0))

    def kmajor(w):  # [K, C] with K=1024 -> [128, 8, C]
        return np.ascontiguousarray(
            w.reshape(8, P, -1).transpose(1, 0, 2)).astype(BF)

    lgT = logits.T  # [1024, 2048]
    whm = kmajor(np.asarray(inputs["head_w"], np.float32))
    w0pm = kmajor(np.asarray(inputs["t0_pw"], np.float32))
    w0pb = np.ascontiguousarray(t0_pb[None, :]).astype(BF)
    w0sm = np.ascontiguousarray(
        np.asarray(inputs["t0_sw"], np.float32).reshape(2, P, T0_SIZE)
        .transpose(1, 0, 2)).astype(BF)
    w0b = np.ascontiguousarray(t0_sb[None, :]).astype(BF)
    w1pm = kmajor(np.asarray(inputs["t1_pw"], np.float32))
    w1pb = np.ascontiguousarray(t1_pb[None, :]).astype(BF)
    t1_sw = np.asarray(inputs["t1_sw"], np.float32)
    if has_sb1:
        w1s_pad = np.zeros((65, T1_PAD), np.float32)
        w1s_pad[:64, :T1_SIZE] = t1_sw
        w1s_pad[64, :T1_SIZE] = t1_sb_arr
        w1s_pad[64, T1_SIZE:] = NEG_BIG
        w1s_bf = w1s_pad.astype(BF)
    else:
        w1s_pk = np.zeros((P, T1_HALF), np.float32)
        w1s_pk[0:64, :] = t1_sw[:, :T1_HALF]
        w1s_pk[64:128, :T1_SIZE - T1_HALF] = t1_sw[:, T1_HALF:]
        w1s_bf = np.ascontiguousarray(w1s_pk).astype(BF)

    m0 = ((targets >= CUT0) & (targets < CUT1)).astype(np.float32)
    m1 = (targets >= CUT1).astype(np.float32)

    in_maps = []
    for i in range(N_CORES):
        sl = slice(i * ROWS, (i + 1) * ROWS)
        lgm = np.ascontiguousarray(
            lgT[:, sl].reshape(8, P, ROWS).transpose(1, 0, 2)).astype(BF)
        cols = []
        for arr in (m0[sl], 1.0 - m0[sl], m1[sl], 1.0 - m1[sl]):
            for rt in range(RT):
                cols.append(arr[rt * P:(rt + 1) * P])
        mkm = np.ascontiguousarray(np.stack(cols, axis=1))  # [128, 8]
        in_maps.append({
            "lgm": lgm, "whm": whm, "w0pm": w0pm, "w0pb": w0pb,
            "w0sm": w0sm, "w0b": w0b, "w1pm": w1pm, "w1pb": w1pb,
            "w1s": w1s_bf, "mkm": mkm,
        })
    return in_maps, m0.astype(bool), m1.astype(bool), (has_pb, has_sb0, has_sb1)


def kernel(**inputs):
    from concourse.bass_utils import run_bass_kernel_spmd

    targets = np.asarray(inputs["targets"], np.int32)
    in_maps, m0b, m1b, flags = prep_in_maps(inputs)
    nc = _build(has_pb=flags[0], has_sb0=flags[1], has_sb1=flags[2])
    res = run_bass_kernel_spmd(nc, in_maps, core_ids=list(range(N_CORES)))
    probs = np.concatenate([res.results[i]["probs"] for i in range(N_CORES)], axis=0)
    aux = np.concatenate([res.results[i]["aux"] for i in range(N_CORES)], axis=0)

    # host-side loss reconstruction (tiny: N gathers + logs)
    p_t = probs[np.arange(N_TOK), targets].astype(np.float64)
    aux64 = aux.astype(np.float64)
    log_p = np.log(p_t)
    root_ce = np.where(m0b, -np.log(aux64[:, 0]),
                       np.where(m1b, -np.log(aux64[:, 1]), -log_p))
    loss = root_ce.sum() / N_TOK
    for i, mb in ((0, m0b), (1, m1b)):
        ce = -(log_p - np.log(aux64[:, i]))
        cnt = max(mb.sum(), 1.0)
        loss += (ce * mb).sum() / cnt
    return probs, np.float32(loss)


# revision 29
# speedup vs baseline: 1.1142x; 1.1142x over previous
"""AdaptiveSoftmax (loss_fn) on 8 TRN2 NeuronCores.

Strategy: data-parallel over the token dim N=2048 -> 256 rows/core.
Each core computes its [256, 50257] slice of probs plus the per-row
cluster probabilities (aux) needed to reconstruct the scalar loss on
host from gathered outputs.

Device program (SPMD, identical on all cores), per 128-row tile:
  root:  e = exp(logitsT.T @ head_w) (bf16 in, fp32 psum, K=1024) with
         fused row-sum -> head probs = e/s, clust_prob = e[:,2000:2002]/s
  tail0: h0 = logits' @ [pw0; pb0]; e0 = exp(h0T.T @ sw0 + sb0) stored
         with row sums; output = e0*A + B on DVE, A = m0*clust0/s0,
         B = (1-m0)*clust0/8000; one 4MB DMA per row-tile.
  tail1: two passes (exp-sum, then fused out = exp(tlog*m1 + bias),
         bias = ln(clust1) - m1*ln(s1) - (1-m1)*ln(40257)), recomputing
         tlog to avoid storing the 20MB intermediate. When sb1 == 0 the
         [64, 40257] weight matrix is packed as two column halves on the
         full 128 partitions (K=64, h1T duplicated on both halves) --
         halves its SBUF footprint and fixes its DMA parallelism; a
         65-row layout with the bias folded as a weight row (ones row in
         h1T) is the fallback. ACT program order weaves tail1 pass2
         (DMA-bound) with the next row-tile's pass1 (compute-only) so
         the scalar engine never idles on the output stream.

Perf notes: inputs are packed on host so each tensor is one or few
DMAs ([128, k, cols] K-major layouts), ordered so root's weights land
first; output DMA issue alternates between the sync and gpsimd
sequencers and outputs are staged 3 ACT-blocks (4608 cols) wide for
better DMA efficiency (measured 305 vs 275 GB/s/core). exp skips
max-subtraction: all softmax inputs stay within +-4 (0.02-scaled
weights) where fp32 exp is 2-ULP exact. ACT ops run on 1536-col
(3 PSUM bank) blocks; pass-1 exp dumps in place into its own PSUM
block. Root weights and the stored e0 share one SBUF region via nested
pools (root finishes before tail0's sums are consumed).
"""
import sys

sys.path.insert(0, "/opt/trn_rl_repo")
import numpy as np

N_CORES = 8
N_TOK, D, V = 2048, 1024, 50257
ROWS = N_TOK // N_CORES          # 256 rows per core
P = 128                          # partitions
RT = ROWS // P                   # 2 row tiles per core
CUT0, CUT1 = 2000, 10000
H_UNITS = 2002                   # head width (2000 + 2 cluster cols)
T0_SIZE = CUT1 - CUT0            # 8000
T1_SIZE = V - CUT1               # 40257
T1_VPAD = 79 * 512               # 40448: pass-1 col coverage (incl pad tail)
T1_PAD = 80 * 512                # 40960: DRAM padding of w1s
NEG_BIG = -1.0e30                # pad-col bias so exp() there is exactly 0
BLK = 1536                       # ACT/psum block width (3 PSUM banks)
T1_HALF = 20480                  # packed-w1s column split (512-aligned)


def _blocks(total, blk=BLK):
    out, c = [], 0
    while c < total:
        out.append((c, min(blk, total - c)))
        c += blk
    return out


def _build(reps=1, phases=("root", "t0", "t1"), has_pb=True, has_sb0=True, has_sb1=True):
    import concourse.bacc as bacc
    import concourse.mybir as mybir
    from concourse import tile
    from concourse.masks import make_identity

    F32 = mybir.dt.float32
    BF16 = mybir.dt.bfloat16

    nc = bacc.Bacc(None, target_bir_lowering=False, debug=True)
    lgm = nc.declare_dram_parameter("lgm", [P, 8, ROWS], BF16, isOutput=False)
    whm = nc.declare_dram_parameter("whm", [P, 8, H_UNITS], BF16, isOutput=False)
    w0pm = nc.declare_dram_parameter("w0pm", [P, 8, 256], BF16, isOutput=False)
    w0pb = nc.declare_dram_parameter("w0pb", [1, 256], BF16, isOutput=False)
    w0sm = nc.declare_dram_parameter("w0sm", [P, 2, T0_SIZE], BF16, isOutput=False)
    w0b = nc.declare_dram_parameter("w0b", [1, T0_SIZE], BF16, isOutput=False)
    w1pm = nc.declare_dram_parameter("w1pm", [P, 8, 64], BF16, isOutput=False)
    w1pb = nc.declare_dram_parameter("w1pb", [1, 64], BF16, isOutput=False)
    if has_sb1:
        w1s = nc.declare_dram_parameter("w1s", [65, T1_PAD], BF16, isOutput=False)
    else:
        # sb1 == 0: two column halves packed onto the full 128 partitions
        w1s = nc.declare_dram_parameter("w1s", [P, T1_HALF], BF16, isOutput=False)
    mkm = nc.declare_dram_parameter("mkm", [P, 8], F32, isOutput=False)
    probs = nc.declare_dram_parameter("probs", [ROWS, V], F32, isOutput=True)
    aux = nc.declare_dram_parameter("aux", [ROWS, 2], F32, isOutput=True)

    with tile.TileContext(nc) as tc:
        for _rep in range(reps):
            _trace_body(nc, tc, mybir, make_identity,
                        lgm, whm, w0pm, w0pb, w0sm, w0b, w1pm, w1pb, w1s, mkm,
                        probs, aux, phases, has_pb, has_sb0, has_sb1)
    nc.compile()
    return nc


def _trace_body(nc, tc, mybir, make_identity,
                lgm, whm, w0pm, w0pb, w0sm, w0b, w1pm, w1pb, w1s, mkm,
                probs, aux, phases, has_pb, has_sb0, has_sb1):
    F32 = mybir.dt.float32
    BF16 = mybir.dt.bfloat16
    Exp = mybir.ActivationFunctionType.Exp
    Ln = mybir.ActivationFunctionType.Ln
    Alu = mybir.AluOpType
    AX = mybir.AxisListType
    import contextlib

    out_eng = [nc.sync, nc.gpsimd]   # alternate output-DMA issuing engine
    out_i = [0]

    def out_dma(dst, src):
        out_eng[out_i[0] % 2].dma_start(out=dst, in_=src)
        out_i[0] += 1

    with (
        contextlib.nullcontext(tc),
        tc.tile_pool(name="const", bufs=1) as constp,
        tc.tile_pool(name="lg", bufs=1) as lgp,
        tc.tile_pool(name="ll", bufs=1) as llp,
        tc.tile_pool(name="wpool", bufs=1) as wpool,
        tc.tile_pool(name="acc", bufs=2) as accp,
        tc.tile_pool(name="rowv", bufs=2) as rowp,
        tc.tile_pool(name="er", bufs=2 if not has_sb1 else 1) as erp,
        tc.tile_pool(name="hsc", bufs=2) as hscp,
        tc.tile_pool(name="stage", bufs=3 if not has_sb1 else 2) as stagep,
        tc.tile_pool(name="ps_main", bufs=2, space="PSUM") as psm,
        tc.tile_pool(name="ps_h", bufs=1, space="PSUM") as psh,
        tc.tile_pool(name="ps_t", bufs=1, space="PSUM") as pst,
    ):
        ident = constp.tile([P, P], BF16, tag="ident")
        make_identity(nc, ident[:])
        ones_row = constp.tile([1, ROWS], BF16, tag="ones_row")
        nc.vector.memset(ones_row[:], 1.0)

        lg = lgp.tile([P, 8, ROWS], BF16, tag="lg")
        nc.gpsimd.dma_start(out=lg[:, :, :], in_=lgm[:, :, :])
        whp_ctx = tc.tile_pool(name="whp", bufs=1)
        whp = whp_ctx.__enter__()
        wh_t = None
        if "root" in phases:
            wh_t = whp.tile([P, 8, H_UNITS], BF16, tag="wh")
            # split so the first root block's columns land ~10us earlier
            nc.gpsimd.dma_start(out=wh_t[:, :, 0:BLK], in_=whm[:, :, 0:BLK])
            nc.gpsimd.dma_start(out=wh_t[:, :, BLK:], in_=whm[:, :, BLK:])

        mk_t = llp.tile([P, 8], F32, tag="mk")
        nc.gpsimd.dma_start(out=mk_t[:, :], in_=mkm[:, :])
        m_t = {}
        for j, nm in enumerate(("m0", "m0c", "m1", "m1c")):
            for rt in range(RT):
                m_t[(nm, rt)] = mk_t[:, j * RT + rt:j * RT + rt + 1]
        clust = [llp.tile([P, 2], F32, tag=f"clust{rt}", name=f"clust{rt}") for rt in range(RT)]

        if "root" not in phases:
            for rt in range(RT):
                nc.vector.memset(clust[rt][:, :], 0.25)

        # non-root weights (root's wh was DMA'd first, above)
        w0p_t = w0pb_t = w0s_t = w0b_t = None
        w1p_t = w1pb_t = w1s_t = None
        if "t0" in phases:
            w0p_t = wpool.tile([P, 8, 256], BF16, tag="w0p")
            nc.gpsimd.dma_start(out=w0p_t[:, :, :], in_=w0pm[:, :, :])
            w0s_t = wpool.tile([P, 2, T0_SIZE], BF16, tag="w0s")
            nc.gpsimd.dma_start(out=w0s_t[:, :, :], in_=w0sm[:, :, :])
            if has_pb:
                w0pb_t = wpool.tile([1, 256], BF16, tag="w0pb")
                nc.gpsimd.dma_start(out=w0pb_t[:, :], in_=w0pb[0:1, :])
            if has_sb0:
                w0b_t = wpool.tile([1, T0_SIZE], BF16, tag="w0b")
                nc.gpsimd.dma_start(out=w0b_t[:, :], in_=w0b[0:1, :])
        if "t1" in phases:
            w1p_t = wpool.tile([P, 8, 64], BF16, tag="w1p")
            nc.gpsimd.dma_start(out=w1p_t[:, :, :], in_=w1pm[:, :, :])
            if has_pb:
                w1pb_t = wpool.tile([1, 64], BF16, tag="w1pb")
                nc.gpsimd.dma_start(out=w1pb_t[:, :], in_=w1pb[0:1, :])
            if has_sb1:
                w1s_t = wpool.tile([65, T1_PAD], BF16, tag="w1s")
                for q in range(4):
                    nc.gpsimd.dma_start(out=w1s_t[:, q * 10240:(q + 1) * 10240],
                                        in_=w1s[:, q * 10240:(q + 1) * 10240])
            else:
                w1s_t = wpool.tile([P, T1_HALF], BF16, tag="w1s")
                nc.gpsimd.dma_start(out=w1s_t[:, :], in_=w1s[:, :])

        def project(rt, width, wp_t, wpb_t, htag):
            """h = logits' @ wp' -> transposed bf16 K-chunk tiles."""
            rs = slice(rt * P, (rt + 1) * P)
            ph = psh.tile([P, 256], F32, tag="ph")
            for k in range(8):
                nc.tensor.matmul(ph[:, :width], lg[:, k, rs], wp_t[:, k, :],
                                 start=(k == 0), stop=(k == 7 and not has_pb))
            if has_pb:
                nc.tensor.matmul(ph[:, :width], ones_row[:, rs], wpb_t[:, :],
                                 start=False, stop=True)
            h = hscp.tile([P, width], BF16, tag=f"h_{htag}")
            nc.vector.tensor_copy(h[:, :], ph[:, :width])
            parts = []
            for c in range(0, width, P):
                cw = min(P, width - c)
                ptt = pst.tile([P, P], BF16, tag="ptt")
                nc.tensor.transpose(ptt[:cw, :], h[:, c:c + cw], ident[:])
                ht = hscp.tile([P, P], BF16, tag=f"hT_{htag}{c // P}")
                nc.vector.tensor_copy(ht[:cw, :], ptt[:cw, :])
                parts.append((ht, cw))
            return parts

        # ---------------- root (nested pool: space reused by e0) ----------
        if True:
            try:
                blocks = _blocks(H_UNITS)
                for rt in (range(RT) if "root" in phases else ()):
                    rs = slice(rt * P, (rt + 1) * P)
                    e_root = erp.tile([P, H_UNITS], F32, tag="e_root")
                    acc_r = accp.tile([P, len(blocks)], F32, tag="acc_r")
                    for bi, (b0, bn) in enumerate(blocks):
                        pt = psm.tile([P, BLK], F32, tag="pm")
                        for s0 in range(0, bn, 512):
                            scn = min(512, bn - s0)
                            for k in range(8):
                                nc.tensor.matmul(
                                    pt[:, s0:s0 + scn], lg[:, k, rs],
                                    wh_t[:, k, b0 + s0:b0 + s0 + scn],
                                    start=(k == 0), stop=(k == 7),
                                )
                        nc.scalar.activation(e_root[:, b0:b0 + bn], pt[:, :bn], Exp,
                                             accum_out=acc_r[:, bi:bi + 1])
                    s_r = rowp.tile([P, 1], F32, tag="s_r")
                    nc.vector.tensor_reduce(s_r[:, :], acc_r[:, :], axis=AX.X, op=Alu.add)
                    rec_r = rowp.tile([P, 1], F32, tag="rec_r")
                    nc.vector.reciprocal(rec_r[:, :], s_r[:, :])
                    nc.vector.tensor_scalar(
                        clust[rt][:, :], e_root[:, CUT0:CUT0 + 2], rec_r[:, :], None, Alu.mult
                    )
                    out_dma(aux[rs, :], clust[rt][:, :])
                    nc.vector.tensor_scalar(e_root[:, 0:CUT0], e_root[:, 0:CUT0],
                                            rec_r[:, :], None, Alu.mult)
                    out_dma(probs[rs, 0:CUT0], e_root[:, 0:CUT0])
            finally:
                whp_ctx.__exit__(None, None, None)

        # ---------------- tails ----------------
        with tc.tile_pool(name="e0p", bufs=1) as e0p:
            def t0_ops(rt):
                """tail0: single exp pass into stored e0; pass2 on DVE."""
                rs = slice(rt * P, (rt + 1) * P)
                hT = project(rt, 256, w0p_t, w0pb_t, "t0")
                blocks = _blocks(T0_SIZE)
                e0 = e0p.tile([P, T0_SIZE], F32, tag="e0", name=f"e0_{rt}")
                acc = accp.tile([P, len(blocks)], F32, tag=f"a0_{rt}",
                                name=f"a0_{rt}")

                def p1_op(bi, b0, bn):
                    def op():
                        pt = psm.tile([P, BLK], F32, tag="pm", name="pt")
                        for s0 in range(0, bn, 512):
                            scn = min(512, bn - s0)
                            c0 = b0 + s0
                            nc.tensor.matmul(pt[:, s0:s0 + scn], hT[0][0][:, :],
                                             w0s_t[:, 0, c0:c0 + scn],
                                             start=True, stop=False)
                            nc.tensor.matmul(pt[:, s0:s0 + scn], hT[1][0][:, :],
                                             w0s_t[:, 1, c0:c0 + scn],
                                             start=False, stop=not has_sb0)
                            if has_sb0:
                                nc.tensor.matmul(pt[:, s0:s0 + scn], ones_row[:, 0:P],
                                                 w0b_t[:, c0:c0 + scn],
                                                 start=False, stop=True)
                        nc.scalar.activation(e0[:, b0:b0 + bn], pt[:, :bn], Exp,
                                             accum_out=acc[:, bi:bi + 1])
                    return op

                def p2():
                    s0_ = rowp.tile([P, 1], F32, tag="s0_", name="s0_")
                    nc.vector.tensor_reduce(s0_[:, :], acc[:, :], axis=AX.X, op=Alu.add)
                    rec0 = rowp.tile([P, 1], F32, tag="rec0", name="rec0")
                    nc.vector.reciprocal(rec0[:, :], s0_[:, :])
                    a0 = rowp.tile([P, 1], F32, tag="a0s", name="a0s")
                    nc.vector.tensor_scalar(a0[:, :], m_t[("m0", rt)],
                                            clust[rt][:, 0:1], rec0[:, :],
                                            Alu.mult, Alu.mult)
                    b0_ = rowp.tile([P, 1], F32, tag="b0s", name="b0s")
                    nc.vector.tensor_scalar(b0_[:, :], m_t[("m0c", rt)],
                                            clust[rt][:, 0:1], 1.0 / T0_SIZE,
                                            Alu.mult, Alu.mult)
                    nc.vector.tensor_scalar(e0[:, :], e0[:, :], a0[:, :], b0_[:, :],
                                            Alu.mult, Alu.add)
                    out_dma(probs[rs, CUT0:CUT1], e0[:, :])

                return [p1_op(bi, b0, bn) for bi, (b0, bn) in enumerate(blocks)], p2

            def t1_ops(rt):
                """tail1: two passes, tlog recomputed; pass2 fused on ACT."""
                rs = slice(rt * P, (rt + 1) * P)
                parts = project(rt, 64, w1p_t, w1pb_t, "t1")
                if has_sb1:
                    h1T = hscp.tile([65, P], BF16, tag=f"h1T_{rt}", name=f"h1T_{rt}")
                    nc.vector.tensor_copy(h1T[0:64, :], parts[0][0][0:64, :])
                    nc.vector.memset(h1T[64:65, :], 1.0)
                    p1_blocks = _blocks(T1_VPAD)
                else:
                    # duplicate h1T on both partition halves for the packed weights
                    h1T = hscp.tile([P, P], BF16, tag=f"h1T_{rt}", name=f"h1T_{rt}")
                    nc.vector.tensor_copy(h1T[0:64, :], parts[0][0][0:64, :])
                    nc.vector.tensor_copy(h1T[64:128, :], parts[0][0][0:64, :])
                    p1_blocks = _blocks(T1_SIZE)
                p2_blocks = _blocks(T1_SIZE)
                acc = accp.tile([P, len(p1_blocks)], F32, tag=f"a1_{rt}",
                                name=f"a1_{rt}")
                uu = rowp.tile([P, 1], F32, tag=f"uu_{rt}", name=f"uu_{rt}")

                def mm_block(pt, b0, bn):
                    for s0 in range(0, bn, 512):
                        scn = min(512, bn - s0)
                        c = b0 + s0
                        if has_sb1:
                            nc.tensor.matmul(pt[:, s0:s0 + scn], h1T[:, :],
                                             w1s_t[:, c:c + scn],
                                             start=True, stop=True)
                        else:
                            lo, qo = (0, c) if c < T1_HALF else (64, c - T1_HALF)
                            nc.tensor.matmul(pt[:, s0:s0 + scn],
                                             h1T[lo:lo + 64, :],
                                             w1s_t[lo:lo + 64, qo:qo + scn],
                                             start=True, stop=True)

                def p1_op(bi, b0, bn):
                    def op():
                        pt = psm.tile([P, BLK], F32, tag="pm", name="pt")
                        mm_block(pt, b0, bn)
                        nc.scalar.activation(pt[:, :bn], pt[:, :bn], Exp,
                                             accum_out=acc[:, bi:bi + 1])
                    return op

                def mid():
                    # u = clust1 * (m1/s1 + (1-m1)/T1): no logs -> the ACT
                    # table never swaps away from the exp set (~2.7us/swap)
                    s1 = rowp.tile([P, 1], F32, tag="s1", name="s1")
                    nc.vector.tensor_reduce(s1[:, :], acc[:, :], axis=AX.X, op=Alu.add)
                    rec1 = rowp.tile([P, 1], F32, tag="rec1", name="rec1")
                    nc.vector.reciprocal(rec1[:, :], s1[:, :])
                    tm = rowp.tile([P, 1], F32, tag="tm", name="tm")
                    nc.vector.tensor_scalar(tm[:, :], m_t[("m1", rt)],
                                            rec1[:, :], None, Alu.mult)
                    tmc = rowp.tile([P, 1], F32, tag="tmc", name="tmc")
                    nc.vector.tensor_scalar(tmc[:, :], m_t[("m1c", rt)],
                                            1.0 / T1_SIZE, None, Alu.mult)
                    nc.vector.tensor_tensor(tm[:, :], tm[:, :], tmc[:, :], Alu.add)
                    nc.vector.tensor_scalar(uu[:, :], tm[:, :],
                                            clust[rt][:, 1:2], None, Alu.mult)

                n_pair = 2 if has_sb1 else 3
                def p2_op(pair):
                    def op():
                        st = stagep.tile([P, n_pair * BLK], F32, tag="st", name="st")
                        off = 0
                        base = pair[0][0]
                        for b0, bn in pair:
                            pt = psm.tile([P, BLK], F32, tag="pm", name="pt")
                            mm_block(pt, b0, bn)
                            nc.scalar.activation(st[:, off:off + bn], pt[:, :bn], Exp,
                                                 scale=m_t[("m1", rt)])
                            off += bn
                        nc.vector.tensor_scalar(st[:, :off], st[:, :off], uu[:, :],
                                                None, Alu.mult)
                        out_dma(probs[rs, CUT1 + base:CUT1 + base + off], st[:, :off])
                    return op

                pairs = [p2_blocks[i:i + n_pair] for i in range(0, len(p2_blocks), n_pair)]
                return ([p1_op(bi, b0, bn) for bi, (b0, bn) in enumerate(p1_blocks)],
                        mid,
                        [p2_op(pair) for pair in pairs])

            def weave(a_ops, b_ops):
                na, nb = len(a_ops), len(b_ops)
                n = max(na, nb)
                ia = ib = 0
                for i in range(n):
                    while ia * n < (i + 1) * na:
                        a_ops[ia]()
                        ia += 1
                    while ib * n < (i + 1) * nb:
                        b_ops[ib]()
                        ib += 1

            do_t0 = "t0" in phases
            do_t1 = "t1" in phases
            t0s = [t0_ops(rt) for rt in range(RT)] if do_t0 else []
            if do_t1:
                t1a_p1, t1a_mid, t1a_p2 = t1_ops(0)
                t1b_p1, t1b_mid, t1b_p2 = t1_ops(1)
            if do_t0:
                for op in t0s[0][0]:
                    op()
                t0s[0][1]()
            if do_t1:
                # rt1's tail0 pass1 hides inside tail1 rt0 pass1 (the shared
                # e0 buffer frees once rt0's 4MB output DMA drains)
                weave(t1a_p1, t0s[1][0] if do_t0 else [])
                if do_t0:
                    t0s[1][1]()
                t1a_mid()
                weave(t1a_p2, t1b_p1)
                t1b_mid()
                for op in t1b_p2:
                    op()
            elif do_t0:
                for op in t0s[1][0]:
                    op()
                t0s[1][1]()


def prep_in_maps(inputs):
    """Host-side prep shared by kernel() and bench: shard + pack + augment."""
    import ml_dtypes
    BF = ml_dtypes.bfloat16

    logits = np.asarray(inputs["logits"], np.float32)
    targets = np.asarray(inputs["targets"], np.int32)
    t0_pb = np.asarray(inputs["t0_pb"], np.float32)
    t1_pb = np.asarray(inputs["t1_pb"], np.float32)
    t0_sb = np.asarray(inputs["t0_sb"], np.float32)
    t1_sb_arr = np.asarray(inputs["t1_sb"], np.float32)
    has_pb = bool(np.any(t0_pb != 0) or np.any(t1_pb != 0))
    has_sb0 = bool(np.any(t0_sb != 0))
    has_sb1 = bool(np.any(t1_sb_arr != 0))

    def kmajor(w):  # [K, C] with K=1024 -> [128, 8, C]
        return np.ascontiguousarray(
            w.reshape(8, P, -1).transpose(1, 0, 2)).astype(BF)

    lgT = logits.T  # [1024, 2048]
    whm = kmajor(np.asarray(inputs["head_w"], np.float32))
    w0pm = kmajor(np.asarray(inputs["t0_pw"], np.float32))
    w0pb = np.ascontiguousarray(t0_pb[None, :]).astype(BF)
    w0sm = np.ascontiguousarray(
        np.asarray(inputs["t0_sw"], np.float32).reshape(2, P, T0_SIZE)
        .transpose(1, 0, 2)).astype(BF)
    w0b = np.ascontiguousarray(t0_sb[None, :]).astype(BF)
    w1pm = kmajor(np.asarray(inputs["t1_pw"], np.float32))
    w1pb = np.ascontiguousarray(t1_pb[None, :]).astype(BF)
    t1_sw = np.asarray(inputs["t1_sw"], np.float32)
    if has_sb1:
        w1s_pad = np.zeros((65, T1_PAD), np.float32)
        w1s_pad[:64, :T1_SIZE] = t1_sw
        w1s_pad[64, :T1_SIZE] = t1_sb_arr
        w1s_pad[64, T1_SIZE:] = NEG_BIG
        w1s_bf = w1s_pad.astype(BF)
    else:
        w1s_pk = np.zeros((P, T1_HALF), np.float32)
        w1s_pk[0:64, :] = t1_sw[:, :T1_HALF]
        w1s_pk[64:128, :T1_SIZE - T1_HALF] = t1_sw[:, T1_HALF:]
        w1s_bf = np.ascontiguousarray(w1s_pk).astype(BF)

    m0 = ((targets >= CUT0) & (targets < CUT1)).astype(np.float32)
    m1 = (targets >= CUT1).astype(np.float32)

    in_maps = []
    for i in range(N_CORES):
        sl = slice(i * ROWS, (i + 1) * ROWS)
        lgm = np.ascontiguousarray(
            lgT[:, sl].reshape(8, P, ROWS).transpose(1, 0, 2)).astype(BF)
        cols = []
        for arr in (m0[sl], 1.0 - m0[sl], m1[sl], 1.0 - m1[sl]):
            for rt in range(RT):
                cols.append(arr[rt * P:(rt + 1) * P])
        mkm = np.ascontiguousarray(np.stack(cols, axis=1))  # [128, 8]
        in_maps.append({
            "lgm": lgm, "whm": whm, "w0pm": w0pm, "w0pb": w0pb,
            "w0sm": w0sm, "w0b": w0b, "w1pm": w1pm, "w1pb": w1pb,
            "w1s": w1s_bf, "mkm": mkm,
        })
    return in_maps, m0.astype(bool), m1.astype(bool), (has_pb, has_sb0, has_sb1)


def kernel(**inputs):
    from concourse.bass_utils import run_bass_kernel_spmd

    targets = np.asarray(inputs["targets"], np.int32)
    in_maps, m0b, m1b, flags = prep_in_maps(inputs)
    nc = _build(has_pb=flags[0], has_sb0=flags[1], has_sb1=flags[2])
    res = run_bass_kernel_spmd(nc, in_maps, core_ids=list(range(N_CORES)))
    probs = np.concatenate([res.results[i]["probs"] for i in range(N_CORES)], axis=0)
    aux = np.concatenate([res.results[i]["aux"] for i in range(N_CORES)], axis=0)

    # host-side loss reconstruction (tiny: N gathers + logs)
    p_t = probs[np.arange(N_TOK), targets].astype(np.float64)
    aux64 = aux.astype(np.float64)
    log_p = np.log(p_t)
    root_ce = np.where(m0b, -np.log(aux64[:, 0]),
                       np.where(m1b, -np.log(aux64[:, 1]), -log_p))
    loss = root_ce.sum() / N_TOK
    for i, mb in ((0, m0b), (1, m1b)):
        ce = -(log_p - np.log(aux64[:, i]))
        cnt = max(mb.sum(), 1.0)
        loss += (ce * mb).sum() / cnt
    return probs, np.float32(loss)
